# revision 1
# baseline (speedup 1.0000x reference)
"""KNN classify kernel for TRN2 (8 NeuronCores).

Strategy: shard X over N (12500 points/core, padded to 12800). Every core
scores all 2048 queries against its shard with a single fused fp32r matmul
(scores s[b,n] = 2*q.x - ||x||^2; the per-row -||q||^2 term is dropped as it
doesn't change per-row ranking, and ||x||^2 enters as 3 extra bf16-split
contraction rows so no elementwise epilogue is needed). The DVE max/max_index
ops extract the top-8 candidates per 2560-wide slab. The host merges the
8x40 candidate lists per query, rescores a small rescue set exactly in fp64,
and emits the label-vote output.
"""

import sys

sys.path.insert(0, "/opt/trn_rl_repo")

import ml_dtypes
import numpy as np

import concourse.bacc as bacc
import concourse.mybir as mybir
from concourse import bass_utils
from concourse.tile import TileContext

B, D, N = 2048, 512, 100000
NCORES = 8
NSH = N // NCORES  # 12500 shard points per core
NPAD = 12800  # 25 * 512
NF = 512  # matmul moving free dim
SLAB = 2560  # top-8 scan window (5 chunks of 512)
NSLABS = NPAD // SLAB  # 5
NCH = SLAB // NF  # 5
P = 128
KC = 5  # contraction chunks: 512 data rows + 3 x2 rows + pad -> 640
KROWS = KC * P
BLK = B // P  # 16
NCAND = NSLABS * 8  # 40 candidates per core per query

_prog = None


def _build_program():
    nc = bacc.Bacc("TRN2", target_bir_lowering=False, debug=False, num_devices=NCORES)
    qt_d = nc.dram_tensor("qt", (KROWS, B), mybir.dt.float32r, kind="ExternalInput")
    xt_d = nc.dram_tensor("xt", (KROWS, NPAD), mybir.dt.float32r, kind="ExternalInput")
    vals_d = nc.dram_tensor("cand_vals", (B, NCAND), mybir.dt.float32, kind="ExternalOutput")
    idx_d = nc.dram_tensor("cand_idx", (B, NCAND), mybir.dt.uint32, kind="ExternalOutput")

    with TileContext(nc) as tc:
        with (
            tc.tile_pool(name="const", bufs=1) as cpool,
            tc.tile_pool(name="xtp", bufs=2) as xpool,
            tc.tile_pool(name="scp", bufs=3) as spool,
            tc.tile_pool(name="psp", bufs=8, space="PSUM") as ppool,
        ):
            # Separate tiles per block / per d-chunk: Tile deps are
            # tile-granular, so this is what lets the first matmuls start
            # after ~1.6 MB of DMA instead of ~12 MB (HAM cold-start fix).
            def load_qt_blk(blk):
                t = cpool.tile([P, KC, P], mybir.dt.float32r, tag=f"qt{blk}", name=f"qt{blk}")
                nc.sync.dma_start(
                    t,
                    qt_d.ap()[:, blk * P : (blk + 1) * P].rearrange(
                        "(c p) b -> p c b", p=P
                    ),
                )
                return t

            def load_xt_chunk(s, d):
                t = xpool.tile([P, SLAB], mybir.dt.float32r, tag=f"xt{d}", name=f"xt{s}_{d}")
                nc.sync.dma_start(
                    t,
                    xt_d.ap()[
                        d * P : (d + 1) * P, s * SLAB : (s + 1) * SLAB
                    ].rearrange("(c p) n -> p c n", p=P),
                )
                return t

            # Warm-up: dummy matmuls with no DMA deps run during the initial
            # ~14us input-DMA wait, so HAM un-throttles before real work.
            warm = cpool.tile([P, NF], mybir.dt.float32, tag="warm", name="warm")
            nc.vector.memset(warm, 0.0)
            wps = ppool.tile([P, NF], mybir.dt.float32, tag="ps", name="wps")
            for _ in range(30):
                nc.tensor.matmul(
                    wps[:, :P], warm[:, :P], warm[:, :P], start=True, stop=True
                )

            qts = [load_qt_blk(0)]
            cv = cpool.tile([P, BLK, NCAND], mybir.dt.float32, tag="cv")
            ci = cpool.tile([P, BLK, NCAND], mybir.dt.uint32, tag="ci")

            xts = [load_xt_chunk(0, d) for d in range(KC)]
            qts += [load_qt_blk(blk) for blk in range(1, BLK)]

            for s in range(NSLABS):
                xt = xts
                if s + 1 < NSLABS:
                    xts = []
                for blk in range(BLK):
                    sc = spool.tile([P, SLAB], mybir.dt.float32, tag="sc")
                    pss = [
                        ppool.tile([P, NF], mybir.dt.float32, tag="ps", name=f"ps{n}")
                        for n in range(NCH)
                    ]
                    for d in range(KC):
                        for n in range(NCH):
                            nc.tensor.matmul(
                                pss[n],
                                qts[blk][:, d, :],
                                xt[d][:, n * NF : (n + 1) * NF],
                                start=(d == 0),
                                stop=(d == KC - 1),
                            )
                    for n in range(NCH):
                        nc.scalar.copy(sc[:, n * NF : (n + 1) * NF], pss[n])
                    mv = cv[:, blk, s * 8 : (s + 1) * 8]
                    nc.vector.max(out=mv, in_=sc)
                    nc.vector.max_index(
                        out=ci[:, blk, s * 8 : (s + 1) * 8], in_max=mv, in_values=sc
                    )
                    # prefetch next slab's chunks spread across early blocks
                    if s + 1 < NSLABS and blk < KC:
                        xts.append(load_xt_chunk(s + 1, blk))

            nc.sync.dma_start(vals_d.ap().rearrange("(blk p) j -> p blk j", p=P), cv)
            nc.sync.dma_start(idx_d.ap().rearrange("(blk p) j -> p blk j", p=P), ci)

    nc.compile()
    return nc


def _prepare_inputs(queries, X):
    queries = np.asarray(queries, np.float32)
    X = np.asarray(X, np.float32)
    qt = np.zeros((KROWS, B), np.float32)
    qt[:D] = 2.0 * queries.T
    qt[D : D + 3] = 1.0

    x2 = (X.astype(np.float64) ** 2).sum(1)
    v = -x2
    p1 = v.astype(ml_dtypes.bfloat16).astype(np.float64)
    p2 = (v - p1).astype(ml_dtypes.bfloat16).astype(np.float64)
    p3 = (v - p1 - p2).astype(np.float32)

    Xt = X.T  # [D, N]
    in_maps = []
    for c in range(NCORES):
        sl = slice(c * NSH, (c + 1) * NSH)
        xt = np.zeros((KROWS, NPAD), np.float32)
        xt[:D, :NSH] = Xt[:, sl]
        xt[D, :NSH] = p1[sl].astype(np.float32)
        xt[D + 1, :NSH] = p2[sl].astype(np.float32)
        xt[D + 2, :NSH] = p3[sl]
        xt[D, NSH:] = -1e30  # padding columns always lose
        in_maps.append({"qt": qt, "xt": xt})
    return in_maps


def _run_device(queries, X, trace=False, trace_kwargs=None):
    global _prog
    if _prog is None:
        _prog = _build_program()
    in_maps = _prepare_inputs(queries, X)
    res = bass_utils.run_bass_kernel_spmd(
        _prog,
        in_maps,
        core_ids=list(range(NCORES)),
        trace=trace,
        **(trace_kwargs or {}),
    )
    return res


def _merge(queries, X, Y, K, res):
    vals = np.stack([res.results[c]["cand_vals"] for c in range(NCORES)])  # [8,B,40]
    idxs = np.stack([res.results[c]["cand_idx"] for c in range(NCORES)]).astype(
        np.int64
    )
    slab_off = (np.arange(NCAND) // 8) * SLAB
    gidx = idxs + slab_off[None, None, :] + (np.arange(NCORES) * NSH)[:, None, None]

    av = vals.transpose(1, 0, 2).reshape(B, NCORES * NCAND)
    ag = gidx.transpose(1, 0, 2).reshape(B, NCORES * NCAND)

    K = int(K)
    rescue = min(max(16, K), NCORES * NCAND)
    sel = np.argpartition(-av, rescue - 1, axis=1)[:, :rescue]
    cand = np.take_along_axis(ag, sel, 1)
    cand = np.clip(cand, 0, N - 1)

    qs = np.asarray(queries, np.float64)
    Xc = np.asarray(X, np.float64)[cand.reshape(-1)].reshape(B, rescue, D)
    d2 = ((Xc - qs[:, None, :]) ** 2).sum(-1)  # [B, rescue]
    order = np.argsort(d2, axis=1, kind="stable")[:, :K]
    top = np.take_along_axis(cand, order, 1)  # [B, K]

    labels = np.asarray(Y)[top].astype(np.float32)
    votes = labels.mean(1)
    out = np.zeros((B, 2), np.float32)
    out[:, 0] = votes
    return out


def kernel(queries, X, Y, K):
    res = _run_device(queries, X)
    return _merge(queries, X, Y, K, res)

